# revision 1
# baseline (speedup 1.0000x reference)
"""Trainium2 Bass kernel for nn_Encoder_79585743995180 (sparse_attention).

v2 — batch x head-group sharding. Core c -> (batch n = c//2, head-group
g = c%2 owning 8 heads / 512 dims). vs v1 (head-only sharding):
  - per-core x DMA drops 24MB -> 6MB (each core reads only its batch);
  - projections/attention/outproj all for 8 heads of one batch;
  - renorm path rebuilt: reciprocal_approx_fast (5x faster than
    InstReciprocal), Pool partition_broadcast instead of DRAM-roundtrip
    broadcast DMAs, no [1,1024] multiplies;
  - matmul stream ordered so the PE never waits on the softmax chain
    (QK/memQK of head h+1 emitted before AV of head h).

All matmul operands fp16 (fp8 validated too lossy: >2e-2). fp32 PSUM.
Math per head (validated in numpy, rel err ~7.7e-4):
  z[s,l] = k_h^T q_h   (q pre-scaled by D^-0.5, rope'd)
  wx = exp(z)
  colsum[s] = sum_l wx ; rcall = 1/colsum
  vs[s,:] = [v_h[s,:] , 1] * rcall   -> AV gives numer[d,l], den[l]
  mem path: wxm = exp(zm), vm cols carry gate*mask, col 64 = mask
  attn_h = numer/den + numer_m/den_m   (division via bc'd reciprocal rows)
out_core[e,l] = sum_dc wo[dc,e] attn[dc,l]; host sums the 2 cores per batch.
"""

import numpy as np

import concourse.bacc as bacc
import concourse.mybir as mybir
import concourse.tile as tile
from concourse import bass_utils

F32 = mybir.dt.float32
F16 = mybir.dt.float16
NPF16 = np.float16
AF = mybir.ActivationFunctionType

L = 1024
S = 1024
N = 4
E = 1024
H = 16
D = 64
M = 512
NC = 8
HPC = 8              # heads per core
DCC = HPC * D        # 512 dims per core

_COMPILED = {}


def _build(dbg=False):
    nc = bacc.Bacc("TRN2", target_bir_lowering=False, debug=False)

    # ---- DRAM I/O (all host-prechunked to [128, ...] partition layouts) ----
    xq = nc.dram_tensor("xq", [128, 8, L], F16, kind="ExternalInput").ap()
    xk = nc.dram_tensor("xk", [128, 8, L], F16, kind="ExternalInput").ap()
    xv = nc.dram_tensor("xv", [128, 8, L], F16, kind="ExternalInput").ap()
    wq = nc.dram_tensor("wq", [128, 8, DCC], F16, kind="ExternalInput").ap()
    wk = nc.dram_tensor("wk", [128, 8, DCC], F16, kind="ExternalInput").ap()
    wv = nc.dram_tensor("wv", [128, 8, DCC], F16, kind="ExternalInput").ap()
    wo = nc.dram_tensor("wo", [128, 4, E], F16, kind="ExternalInput").ap()
    cosq = nc.dram_tensor("cosq", [128, 4, L], F16, kind="ExternalInput").ap()
    sinq = nc.dram_tensor("sinq", [128, 4, L], F16, kind="ExternalInput").ap()
    cosk = nc.dram_tensor("cosk", [128, 4, L], F16, kind="ExternalInput").ap()
    sink = nc.dram_tensor("sink", [128, 4, L], F16, kind="ExternalInput").ap()
    kmem = nc.dram_tensor("kmem", [128, 4, M], F16, kind="ExternalInput").ap()
    vm = nc.dram_tensor("vm", [128, HPC, 4, 65], F16, kind="ExternalInput").ap()
    outT = nc.dram_tensor("outT", [128, 8, L], F16, kind="ExternalOutput").ap()
    dbg_t = {}
    if dbg:
        for nm, shp, dt in (("dbg_q", [128, 4, L], F16),
                            ("dbg_k", [128, 4, L], F16),
                            ("dbg_v", [128, 8, HPC, 65], F16),
                            ("dbg_attn", [128, 4, L], F16),
                            ("dbg_colsum", [128, HPC, 8], F32),
                            ("dbg_rcall", [128, HPC, 8], F32),
                            ("dbg_r1", [1, HPC, L], F32),
                            ("dbg_r2", [1, HPC, L], F32),
                            ("dbg_pmain", [65, L], F32),
                            ("dbg_pmem", [65, L], F32),
                            ("dbg_wx", [128, L], F16)):
            dbg_t[nm] = nc.dram_tensor(nm, shp, dt, kind="ExternalOutput").ap()

    with tile.TileContext(nc) as tc:
        with (
            tc.tile_pool(name="const", bufs=1) as const,
            tc.tile_pool(name="persist", bufs=1) as persist,
            tc.tile_pool(name="wexp", bufs=9) as wexpp,
            tc.tile_pool(name="cs", bufs=1) as csp,
            tc.tile_pool(name="small", bufs=16) as small,
            tc.tile_pool(name="vsp", bufs=12) as vsp,
            tc.tile_pool(name="rows", bufs=1) as rows,
            tc.tile_pool(name="bcp", bufs=1) as bcp,
            tc.tile_pool(name="uscr", bufs=1) as uscr,
            tc.tile_pool(name="rscr", bufs=2) as rscr,
            tc.tile_pool(name="ostage", bufs=2) as ostage,
            tc.tile_pool(name="pq", bufs=2, space="PSUM") as pq,
            tc.tile_pool(name="pmain", bufs=1, space="PSUM") as pmainp,
            tc.tile_pool(name="pmem", bufs=1, space="PSUM") as pmemp,
        ):
            # ---- constants / activations into SBUF ----
            # dma_start ISSUE costs ~0.7us of sequencer time each, so the
            # loads are spread across the sync/scalar/gpsimd sequencers
            # (idle at startup) in first-use order: sync carries the q/v
            # path, scalar the k path, gpsimd the late tensors.
            cs_src = {"cq": cosq, "sq": sinq, "ck": cosk, "sk": sink}
            w_sb = {}
            x_sb = {}
            for name, wsrc, xsrc, eng in (("q", wq, xq, nc.sync),
                                          ("v", wv, xv, nc.sync),
                                          ("k", wk, xk, nc.scalar)):
                wt = const.tile([128, 8, DCC], F16, tag=f"w_{name}")
                eng.dma_start(out=wt[0:64], in_=wsrc[0:64])
                eng.dma_start(out=wt[64:128], in_=wsrc[64:128])
                w_sb[name] = wt
                xt = const.tile([128, 8, L], F16, tag=f"x_{name}")
                for kc in range(8):
                    eng.dma_start(out=xt[:, kc, :], in_=xsrc[:, kc, :])
                x_sb[name] = xt
            kmem_sb = const.tile([128, 4, M], F16, tag="kmem")
            nc.scalar.dma_start(out=kmem_sb, in_=kmem)
            vm_sb = const.tile([128, HPC, 4, 65], F16, tag="vm")
            nc.gpsimd.dma_start(out=vm_sb, in_=vm)
            wo_sb = const.tile([128, 4, E], F16, tag="wo")

            # per-dim-group tiles: dependency tracking is tile-granular, so
            # a shared tile would serialize readers against unrelated
            # writers (outproj dg0 would wait on head 7's combine, rope
            # adds on earlier heads' QK reads)
            qT = [persist.tile([128, L], F16, tag=f"qT{i}", name=f"qT{i}")
                  for i in range(4)]
            kT = [persist.tile([128, L], F16, tag=f"kT{i}", name=f"kT{i}")
                  for i in range(4)]
            v16 = persist.tile([128, 8, HPC, 65], F16, tag="v16")
            attn = [persist.tile([128, L], F16, tag=f"at{i}", name=f"at{i}")
                    for i in range(4)]
            nc.vector.memset(v16[:, :, :, 64:65], 1.0)

            class ProjG:
                """One q-or-k projection group (dim-group dg): 2 chunks of
                8 matmuls, each into a short-lived psum tile that a DVE copy
                frees immediately (keeps the shared pq ring flowing when
                chunks are interleaved into the attention stream as gate-free
                PE fill work), then rope on the f16 copy."""

                def __init__(self, dg, name):
                    self.dg, self.name = dg, name
                    self.qraw = rscr.tile([128, L], F16, tag="qraw")
                    self.ct = csp.tile([128, L], F16, tag=f"c{name}")
                    self.st = csp.tile([128, L], F16, tag=f"s{name}")
                    eng = nc.sync if dg == 0 else nc.gpsimd
                    eng.dma_start(
                        out=self.ct,
                        in_=cs_src["cq" if name == "q" else "ck"][:, dg, :])
                    eng.dma_start(
                        out=self.st,
                        in_=cs_src["sq" if name == "q" else "sk"][:, dg, :])

                def chunk(self, lc):
                    ls = slice(lc * 512, (lc + 1) * 512)
                    ps = pq.tile([128, L], F32, tag="pq")
                    for kc in range(8):
                        nc.tensor.matmul(
                            ps[:, 0:512],
                            w_sb[self.name][:, kc,
                                            self.dg * 128:(self.dg + 1) * 128],
                            x_sb[self.name][:, kc, ls],
                            start=(kc == 0), stop=(kc == 7))
                    nc.vector.tensor_copy(self.qraw[:, ls], ps[:, 0:512])

                def finish(self):
                    dest = (qT if self.name == "q" else kT)[self.dg]
                    t1 = rscr.tile([128, L], F16, tag="t1")
                    nc.vector.tensor_mul(t1, self.qraw, self.ct)
                    # z = qraw * sin (sign-folded AND pre-swapped on host);
                    # t2 = partner-swap of z via Pool DMA copies
                    z = rscr.tile([128, L], F16, tag="z")
                    nc.vector.tensor_mul(z, self.qraw, self.st)
                    t2 = rscr.tile([128, L], F16, tag="t2")
                    for a in (0, 64):
                        nc.gpsimd.dma_start(
                            out=t2[a:a + 32, :], in_=z[a + 32:a + 64, :])
                        nc.gpsimd.dma_start(
                            out=t2[a + 32:a + 64, :], in_=z[a:a + 32, :])
                    nc.vector.tensor_add(dest, t1, t2)

            def emit_projqk(dg):
                for name in ("q", "k"):
                    g = ProjG(dg, name)
                    g.chunk(0)
                    g.chunk(1)
                    g.finish()

            def emit_projv():
                # v projection: all 512 dims at once, [s-rows, dims] layout;
                # two row-blocks share one [128, 1024] psum tile
                for sp in range(4):
                    ps = pq.tile([128, L], F32, tag="pq")
                    for half in range(2):
                        st_i = sp * 2 + half
                        hs = slice(half * 512, (half + 1) * 512)
                        for kc in range(8):
                            nc.tensor.matmul(
                                ps[:, hs],
                                x_sb["v"][:, kc, st_i * 128:(st_i + 1) * 128],
                                w_sb["v"][:, kc, :],
                                start=(kc == 0), stop=(kc == 7))
                        for h in range(HPC):
                            if h % 2 == 0:
                                nc.scalar.copy(
                                    v16[:, st_i, h, 0:64],
                                    ps[:, half * 512 + h * 64:
                                       half * 512 + (h + 1) * 64])
                            else:
                                nc.vector.tensor_copy(
                                    v16[:, st_i, h, 0:64],
                                    ps[:, half * 512 + h * 64:
                                       half * 512 + (h + 1) * 64])

            def emit_qk_sc(st, sc):
                # one [128,1024] logits tile + exp for head h, s-chunk sc;
                # the per-chunk reciprocal + v-prescale follow immediately
                # (per-chunk colsum tiles keep the DVE chain off the
                # tile-granular dependency path of later exps), so the AV
                # stationaries are ready the moment the next period starts
                h = st.h
                dg, ho = h // 2, (h % 2) * 64
                pw = pq.tile([128, L], F32, tag="pq")
                for lc in range(2):
                    nc.tensor.matmul(
                        pw[:, lc * 512:(lc + 1) * 512],
                        kT[dg][ho:ho + 64, sc * 128:(sc + 1) * 128],
                        qT[dg][ho:ho + 64, lc * 512:(lc + 1) * 512],
                        start=True, stop=True)
                wx = wexpp.tile([128, L], F16, tag="wx")
                cs1 = small.tile([128, 1], F32, tag="cs1")
                nc.scalar.activation(wx, pw, AF.Exp, accum_out=cs1)
                rc = small.tile([128, 1], F32, tag="rc")
                nc.vector.reciprocal_approx_fast(out=rc, in_=cs1)
                vs = vsp.tile([128, 65], F16, tag="vs")
                nc.vector.tensor_scalar_mul(vs, v16[:, sc, h, :], rc)
                st.wxs.append(wx)
                st.vss.append(vs)

            def emit_memqk_mc(h, mc):
                dg, ho = h // 2, (h % 2) * 64
                pw = pq.tile([128, L], F32, tag="pq")
                for lc in range(2):
                    nc.tensor.matmul(
                        pw[:, lc * 512:(lc + 1) * 512],
                        kmem_sb[ho:ho + 64, dg, mc * 128:(mc + 1) * 128],
                        qT[dg][ho:ho + 64, lc * 512:(lc + 1) * 512],
                        start=True, stop=True)
                wx = wexpp.tile([128, L], F16, tag="wx")
                nc.scalar.activation(wx, pw, AF.Exp)
                return wx

            class Head:
                def __init__(self, h):
                    self.h = h
                    self.wxs = []
                    self.wxm = []
                    self.vss = []
                    self.pmain = None
                    self.pmem = None

            def emit_alloc_psum(st):
                st.pmain = pmainp.tile([65, L], F32, tag="pmain")
                st.pmem = pmemp.tile([65, L], F32, tag="pmem")

            def emit_av_sc(st, sc):
                for lc in range(2):
                    nc.tensor.matmul(
                        st.pmain[:, lc * 512:(lc + 1) * 512],
                        st.vss[sc], st.wxs[sc][:, lc * 512:(lc + 1) * 512],
                        start=(sc == 0), stop=(sc == 7))

            def emit_memav_mc(st, mc):
                for lc in range(2):
                    nc.tensor.matmul(
                        st.pmem[:, lc * 512:(lc + 1) * 512],
                        vm_sb[:, st.h, mc, :],
                        st.wxm[mc][:, lc * 512:(lc + 1) * 512],
                        start=(mc == 0), stop=(mc == 3))

            def emit_release(st):
                # copies free pmain early (the PE's in-order queue must not
                # stall on a WAR against the combine chain); den row goes to
                # its own partition-0 tile: the custom-DVE reciprocal and
                # partition_broadcast require partition-0-based inputs, and
                # TensorTensor partition bases must be 32-aligned
                st.cd1 = rows.tile([1, L], F32, tag="cd1")
                nc.vector.tensor_copy(st.cd1, st.pmain[64:65, :])
                st.np1 = uscr.tile([64, L], F32, tag="np1")
                nc.vector.tensor_copy(st.np1, st.pmain[0:64, :])

            def emit_combine(st):
                # attn_h = np1[:64]/den1 + np2[:64]/den2; den rows are in
                # SBUF (custom-DVE reciprocal bit ops need raw fp32, which
                # PSUM reads would convert)
                h = st.h
                dg, ho = h // 2, (h % 2) * 64
                cd2 = rows.tile([1, L], F32, tag="cd2")
                nc.vector.tensor_copy(cd2, st.pmem[64:65, :])
                np2 = uscr.tile([64, L], F32, tag="np2")
                nc.vector.tensor_copy(np2, st.pmem[0:64, :])
                np1 = st.np1
                r1 = rows.tile([1, L], F32, tag="r1")
                nc.vector.reciprocal_approx_fast(out=r1, in_=st.cd1)
                r2 = rows.tile([1, L], F32, tag="r2")
                nc.vector.reciprocal_approx_fast(out=r2, in_=cd2)
                bc1 = bcp.tile([64, L], F32, tag="bc1")
                nc.gpsimd.partition_broadcast(bc1, r1)
                bc2 = bcp.tile([64, L], F32, tag="bc2")
                nc.gpsimd.partition_broadcast(bc2, r2)
                u1 = uscr.tile([64, L], F16, tag="u1")
                nc.vector.tensor_mul(u1, np1, bc1)
                u2 = uscr.tile([64, L], F16, tag="u2")
                nc.vector.tensor_mul(u2, np2, bc2)
                nc.vector.tensor_add(attn[dg][ho:ho + 64, :], u1, u2)
                if dbg:
                    nc.sync.dma_start(out=dbg_t["dbg_r1"][:, h, :], in_=r1)
                    nc.sync.dma_start(out=dbg_t["dbg_r2"][:, h, :], in_=r2)
                    if h == 0:
                        nc.sync.dma_start(out=dbg_t["dbg_pmain"][0:64], in_=np1)
                        nc.sync.dma_start(out=dbg_t["dbg_pmain"][64:65],
                                          in_=st.cd1)
                        nc.sync.dma_start(out=dbg_t["dbg_pmem"][0:64], in_=np2)
                        nc.sync.dma_start(out=dbg_t["dbg_pmem"][64:65], in_=cd2)
                        nc.sync.dma_start(out=dbg_t["dbg_wx"], in_=st.wxs[0])

            def emit_outproj():
                # first two oc groups emit their dg0-2 partials before any
                # dg3 matmul so the PE has fill work while the last head's
                # combine chain (which dg3 needs) drains
                def mm(po, oc, lc, dg):
                    nc.tensor.matmul(
                        po[:, lc * 512:(lc + 1) * 512],
                        wo_sb[:, dg, oc * 128:(oc + 1) * 128],
                        attn[dg][:, lc * 512:(lc + 1) * 512],
                        start=(dg == 0), stop=(dg == 3))

                def stage_out(po, oc):
                    so = ostage.tile([128, L], F16, tag="so")
                    if oc % 2 == 0:
                        nc.vector.tensor_copy(so, po)
                    else:
                        nc.scalar.copy(so, po)
                    nc.sync.dma_start(out=outT[:, oc, :], in_=so)

                po0 = pq.tile([128, L], F32, tag="pq")
                for lc in range(2):
                    for dg in range(3):
                        mm(po0, 0, lc, dg)
                po1 = pq.tile([128, L], F32, tag="pq")
                for lc in range(2):
                    for dg in range(3):
                        mm(po1, 1, lc, dg)
                for lc in range(2):
                    mm(po0, 0, lc, 3)
                stage_out(po0, 0)
                for lc in range(2):
                    mm(po1, 1, lc, 3)
                stage_out(po1, 1)
                for oc in range(2, 8):
                    po = pq.tile([128, L], F32, tag="pq")
                    for lc in range(2):
                        for dg in range(4):
                            mm(po, oc, lc, dg)
                    stage_out(po, oc)

            # ---- emission ----
            # dg0 + v projections upfront (dense accumulation while input
            # DMA lands); the remaining projection groups are spread through
            # the attention periods as gate-free PE fill work, so the PE
            # in-order queue always has a ready matmul while the scalar
            # engine works through the exp chain (otherwise the tensor
            # clock keeps dropping out of its high p-state). Ungated AV(h)
            # matmuls are emitted BEFORE the exp-gated QK(h+1) tile of the
            # same step for the same reason.
            g0q = ProjG(0, "q")
            g0q.chunk(0)
            g0q.chunk(1)
            g0q.finish()
            emit_projv()
            g0k = ProjG(0, "k")
            g0k.chunk(0)
            g0k.chunk(1)
            g0k.finish()

            nc.gpsimd.dma_start(out=wo_sb[0:64], in_=wo[0:64])
            nc.gpsimd.dma_start(out=wo_sb[64:128], in_=wo[64:128])
            fills = [ProjG(1, "q"), ProjG(1, "k"), ProjG(2, "q"),
                     ProjG(2, "k"), ProjG(3, "q"), ProjG(3, "k")]

            cur = Head(0)
            g = fills.pop(0)
            for sc in range(8):
                if sc == 2:
                    g.chunk(0)
                if sc == 6:
                    g.chunk(1)
                emit_qk_sc(cur, sc)
            for mc in range(4):
                cur.wxm.append(emit_memqk_mc(0, mc))
            g.finish()

            for h in range(HPC):
                nxt = Head(h + 1) if h + 1 < HPC else None
                g = fills.pop(0) if fills else None
                emit_alloc_psum(cur)
                for sc in range(8):
                    emit_av_sc(cur, sc)
                    if g is not None and sc == 2:
                        g.chunk(0)
                    if g is not None and sc == 6:
                        g.chunk(1)
                    if nxt is not None:
                        emit_qk_sc(nxt, sc)
                emit_release(cur)
                for mc in range(4):
                    emit_memav_mc(cur, mc)
                    if nxt is not None:
                        nxt.wxm.append(emit_memqk_mc(h + 1, mc))
                if g is not None:
                    g.finish()
                emit_combine(cur)
                cur = nxt
            if dbg:
                for i in range(4):
                    nc.sync.dma_start(out=dbg_t["dbg_q"][:, i, :], in_=qT[i])
                    nc.sync.dma_start(out=dbg_t["dbg_k"][:, i, :], in_=kT[i])
                    nc.sync.dma_start(out=dbg_t["dbg_attn"][:, i, :],
                                      in_=attn[i])
                nc.sync.dma_start(out=dbg_t["dbg_v"], in_=v16)
            emit_outproj()
    nc.compile()
    return nc


def _perm64():
    p = np.empty(64, np.int64)
    p[:32] = np.arange(0, 64, 2)
    p[32:] = np.arange(1, 64, 2)
    return p


def _chunk(a, nchunk):
    # [C*128, F] -> [128, C, F]
    c128, f = a.shape
    return np.ascontiguousarray(
        a.reshape(nchunk, 128, f).transpose(1, 0, 2)).astype(NPF16)


def _prep_inputs(inputs):
    """Host-side shard prep. Returns list of per-core input dicts."""
    f = np.float32
    query = np.asarray(inputs["query"], f)
    key = np.asarray(inputs["key"], f)
    value = np.asarray(inputs["value"], f)
    W = np.asarray(inputs["in_proj_weight"], f)
    wo = np.asarray(inputs["out_proj_weight"], f)
    qp = np.asarray(inputs["qp"], f)
    kvp = np.asarray(inputs["kvp"], f)
    k_mem = np.asarray(inputs["k_mem"], f)
    v_mem = np.asarray(inputs["v_mem"], f)
    gate = np.asarray(inputs["gate_attn"], f)
    mask = np.asarray(inputs["mem_mask"]).astype(f)

    g = 1.0 / (1.0 + np.exp(-gate))
    p64 = _perm64()
    sgn = np.tile(np.concatenate(
        [np.full(32, -1.0, f), np.full(32, 1.0, f)]), HPC)

    # per-batch x, shared by the two cores of each batch
    xs = {}
    for n in range(N):
        xs[n] = tuple(
            _chunk(np.ascontiguousarray(t[:, n, :].T), 8)
            for t in (query, key, value))

    def swap32(x):
        y = np.empty_like(x)
        for hb in range(HPC):
            b = hb * 64
            y[b:b + 32] = x[b + 32:b + 64]
            y[b + 32:b + 64] = x[b:b + 32]
        return y

    in_maps = []
    for c in range(NC):
        n, grp = c // 2, c % 2
        dims = np.arange(grp * DCC, (grp + 1) * DCC)
        dims_perm = np.concatenate([dims[h * 64 + p64] for h in range(HPC)])
        gv = np.concatenate(
            [np.full(64, 1.0 - g[grp * HPC + h], f) for h in range(HPC)])

        wq_c = _chunk(np.ascontiguousarray(
            (W[:E][dims_perm] * np.float32(D ** -0.5)).T), 8)
        wk_c = _chunk(np.ascontiguousarray(W[E:2 * E][dims_perm].T), 8)
        wv_c = _chunk(np.ascontiguousarray(
            (W[2 * E:][dims] * gv[:, None]).T), 8)
        wo_c = _chunk(np.ascontiguousarray(wo[:, dims].T), 4)

        cq = _chunk(np.ascontiguousarray(qp[n][:, dims_perm, 0].T), 4)
        sq = _chunk(swap32(qp[n][:, dims_perm, 1].T * sgn[:, None]), 4)
        ck = _chunk(np.ascontiguousarray(kvp[n][:, dims_perm, 0].T), 4)
        sk = _chunk(swap32(kvp[n][:, dims_perm, 1].T * sgn[:, None]), 4)

        km = _chunk(np.ascontiguousarray(k_mem[n][dims_perm, :]), 4)

        vma = np.empty((HPC, 4, 128, 65), f)
        for h in range(HPC):
            vmh = (v_mem[n][dims[h * 64:(h + 1) * 64], :].T
                   * g[grp * HPC + h] * mask[n][:, None])      # [M, 64]
            vma[h, :, :, :64] = vmh.reshape(4, 128, 64)
            vma[h, :, :, 64] = mask[n].reshape(4, 128)
        vm_dev = np.ascontiguousarray(
            vma.transpose(2, 0, 1, 3)).astype(NPF16)           # [128,H,4,65]

        xq_c, xk_c, xv_c = xs[n]
        in_maps.append({
            "xq": xq_c, "xk": xk_c, "xv": xv_c,
            "wq": wq_c, "wk": wk_c, "wv": wv_c, "wo": wo_c,
            "cosq": cq, "sinq": sq, "cosk": ck, "sink": sk,
            "kmem": km, "vm": vm_dev,
        })
    return in_maps


def kernel(**inputs):
    if "nc" not in _COMPILED:
        _COMPILED["nc"] = _build()
    nc = _COMPILED["nc"]
    in_maps = _prep_inputs(inputs)
    res = bass_utils.run_bass_kernel_spmd(nc, in_maps, core_ids=list(range(NC)))
    out = np.zeros((L, N, E), np.float64)
    for c, r in enumerate(res.results):
        n = c // 2
        oc = r["outT"].astype(np.float64)          # [128, 8, L]
        out[:, n, :] += oc.transpose(2, 1, 0).reshape(L, E)
    out = out.astype(np.float32) + np.asarray(inputs["out_proj_bias"],
                                              np.float32)
    return out

